# revision 1
# baseline (speedup 1.0000x reference)
"""Trainium2 Bass kernel for DiffKS (differentiable Karplus-Strong string).

Math (per sequence b, time n):
    g = 0.99*l_b[...,0]; p = l_b[...,1]
    b0 = g*(1-p); a1 = g*p
    f0c = f0 - a1/(b0+a1+1e-7)
    z = floor(f0c); zc = z-2; alpha = f0c - zc
    w_j = Lagrange weights (order 5), j=0..5
    block_j = b0*w_j + a1*w_{j-1}, j=0..6           (7 taps)
    taps live at k = c0+j, c0 = zc-1 = z-3 in [36, 96]
    y[n] = x[n] + sum_j block_j[n] * y[n-1-(c0[n]+j)]    (delays 37..103)

Key structure: minimum delay is 37 > 32, so 32-sample chunks are internally
parallel.  Chunk c is computed as 4 accumulating 32x32 PE matmuls against the
previous 4 chunks' outputs, with per-chunk tap matrices built on-chip by a
GPSIMD local_scatter + DVE 32x32 block transpose.  The B=16 batch is sharded
2 sequences per NeuronCore across 8 cores (pure data parallel).

PE constraint: stationary and moving operands must share the same partition
base quadrant, so ALL per-seq data (taps, y ring, x, psum) for seq b lives at
partitions [32b, 32b+32).

Layouts (per core, seqs b=0,1; chunk T=32; NCH = N/32 chunks; NP = N/128):
  natural plane  nat[P, b*128+j]  = q[b, 128*P + j]          [NP, 256]
  S-plane        qS[32b+f, c]     = q[b, 32*c + f]           [64, NCH]
  y ring         ytile[32b+f, 4+c] = y[b, 32*c + f]          [64, 4+NCH]
     (columns 0..3 are zeros = initial state)
Tap matrix for chunk c, source chunk c-q (q=1..4):
  S_q[p, f] = tapval at A[32c+f, 32q-1+f-p]     (lhsT for the PE matmul)
Scatter (groups of G=4 chunks, slot t=c%4): tap (j) of sample (b,c,f) goes to
  Traw[32b+f, 256t + 2*p' (+1)] (u16 pairs), p' = 32*fl + 31 - m,
  v = c0+j-f, fl = v//32 (=q-1), m = v%32 (p = 31-m);
then one DVE 32x32-block transpose gives
  T4[32b+p, 128t + 32(q-1) + f] = S_q^{(c)}[p, f].
"""

import numpy as np

import concourse.bass as bass
import concourse.mybir as mybir
import concourse.bacc as bacc
import concourse.tile as tile
from concourse import bass_utils

F32 = mybir.dt.float32
I32 = mybir.dt.int32
I16 = mybir.dt.int16
U16 = mybir.dt.uint16
AO = mybir.AluOpType
AF = mybir.ActivationFunctionType

B_FULL = 16
N_FULL = 16384
NCORES = 8
B_LOC = 2  # sequences per core
GS = 8     # chunks per scatter group

# matmul piece tables per t=c%4: (row_base, row_size, col_delta); the global
# ring column read is (c//4) + col_delta
PIECES = {
    0: [(0, 128, 0)],
    1: [(32, 32, 0), (64, 64, 0), (0, 32, 1)],
    2: [(64, 64, 0), (0, 64, 1)],
    3: [(96, 32, 0), (0, 64, 1), (64, 32, 1)],
}

# Lagrange denominators 1/d_j for order 5
INV_D = [-1.0 / 120, 1.0 / 24, -1.0 / 12, 1.0 / 12, -1.0 / 24, 1.0 / 120]


def build_kernel(tc, out_d, f0_d, x_d, lb_d, N):
    nc = tc.nc
    NP = N // 128          # natural-plane columns per seq
    NCH = N // 32          # chunks per seq
    NG = NCH // GS         # scatter groups
    assert NP * 128 == N and NP <= 128 and NG * GS == NCH and NP % 2 == 0

    import contextlib
    ctx = contextlib.ExitStack()
    pp = ctx.enter_context(tc.tile_pool(name="persist", bufs=1))
    traw_pool = ctx.enter_context(tc.tile_pool(name="traw", bufs=4))
    t4_pool = ctx.enter_context(tc.tile_pool(name="t4", bufs=16))
    psum_tr = ctx.enter_context(tc.tile_pool(name="psum_tr", bufs=2, space="PSUM"))
    psum_c = ctx.enter_context(tc.tile_pool(name="psum_c", bufs=3, space="PSUM"))

    with ctx:
        # ---------------- phase 0: load + elementwise tap math ----------------
        nat_f0 = pp.tile([NP, 256], F32)
        nat_x = pp.tile([NP, 256], F32)
        nat_lb = pp.tile([NP, 512], F32)
        for b in range(B_LOC):
            nc.sync.dma_start(
                out=nat_f0[:, b * 128:(b + 1) * 128],
                in_=f0_d[b].rearrange("(p j) -> p j", j=128),
            )
            nc.sync.dma_start(
                out=nat_x[:, b * 128:(b + 1) * 128],
                in_=x_d[b].rearrange("(p j) -> p j", j=128),
            )
            nc.sync.dma_start(
                out=nat_lb[:, b * 256:(b + 1) * 256],
                in_=lb_d[b].rearrange("(p j) s -> p (j s)", j=128),
            )
        # strided views of l_b: even cols = g, odd cols = p
        lb_r = nat_lb[:].rearrange("p (j s) -> p j s", s=2)
        g_ap = lb_r[:, :, 0]  # [NP, 256] stride-2
        p_ap = lb_r[:, :, 1]

        g99 = pp.tile([NP, 256], F32)
        t_gp = pp.tile([NP, 256], F32)   # a1 = 0.99*g*p
        b0t = pp.tile([NP, 256], F32)
        rec = pp.tile([NP, 256], F32)
        f0c = pp.tile([NP, 256], F32)
        zf = pp.tile([NP, 256], F32)
        tmp1 = pp.tile([NP, 256], F32)
        tmp2 = pp.tile([NP, 256], F32)
        itmp = pp.tile([NP, 256], I32)

        V = nc.vector
        V.tensor_scalar(out=g99[:], in0=g_ap, scalar1=0.99, scalar2=None, op0=AO.mult)
        V.tensor_tensor(out=t_gp[:], in0=g99[:], in1=p_ap, op=AO.mult)      # a1
        V.tensor_tensor(out=b0t[:], in0=g99[:], in1=t_gp[:], op=AO.subtract)  # b0
        V.tensor_scalar(out=tmp1[:], in0=g99[:], scalar1=1e-7, scalar2=None, op0=AO.add)
        V.reciprocal(out=rec[:], in_=tmp1[:])
        V.tensor_tensor(out=tmp2[:], in0=t_gp[:], in1=rec[:], op=AO.mult)   # a1/(b0+a1+eps)
        V.tensor_tensor(out=f0c[:], in0=nat_f0[:], in1=tmp2[:], op=AO.subtract)
        # zf = floor(f0c), robust to cast rounding mode
        V.tensor_copy(out=itmp[:], in_=f0c[:])
        V.tensor_copy(out=zf[:], in_=itmp[:])
        V.tensor_tensor(out=tmp1[:], in0=zf[:], in1=f0c[:], op=AO.is_gt)
        V.tensor_tensor(out=zf[:], in0=zf[:], in1=tmp1[:], op=AO.subtract)
        # D = f0c - zf  (alpha = D + 2);  u_m = D + (2 - m), m = 0..5
        D = f0c
        V.tensor_tensor(out=D[:], in0=f0c[:], in1=zf[:], op=AO.subtract)

        u = [pp.tile([NP, 256], F32, name=f"u{m}", tag=f"u{m}") for m in range(6)]
        for m in range(6):
            V.tensor_scalar(out=u[m][:], in0=D[:], scalar1=float(2 - m),
                            scalar2=None, op0=AO.add)
        # prefix[j] = u0*..*u_{j-1}, suffix[j] = u_j*..*u5
        pre = [None] * 6
        suf = [None] * 7
        pre[1] = u[0]
        for j in range(2, 6):
            pre[j] = pp.tile([NP, 256], F32, name=f"pre{j}", tag=f"pre{j}")
            V.tensor_tensor(out=pre[j][:], in0=pre[j - 1][:], in1=u[j - 1][:], op=AO.mult)
        suf[5] = u[5]
        for j in range(4, 0, -1):
            suf[j] = pp.tile([NP, 256], F32, name=f"suf{j}", tag=f"suf{j}")
            V.tensor_tensor(out=suf[j][:], in0=suf[j + 1][:], in1=u[j][:], op=AO.mult)
        w = [pp.tile([NP, 256], F32, name=f"w{j}", tag=f"w{j}") for j in range(6)]
        V.tensor_scalar(out=w[0][:], in0=suf[1][:], scalar1=INV_D[0], scalar2=None, op0=AO.mult)
        for j in range(1, 5):
            V.scalar_tensor_tensor(out=w[j][:], in0=pre[j][:], scalar=INV_D[j],
                                   in1=suf[j + 1][:], op0=AO.mult, op1=AO.mult)
        V.tensor_scalar(out=w[5][:], in0=pre[5][:], scalar1=INV_D[5], scalar2=None, op0=AO.mult)

        # block_j = b0*w_j + a1*w_{j-1}, j=0..6
        blk = [pp.tile([NP, 256], F32, name=f"blk{j}", tag=f"blk{j}") for j in range(7)]
        V.tensor_tensor(out=blk[0][:], in0=b0t[:], in1=w[0][:], op=AO.mult)
        for j in range(1, 6):
            V.tensor_tensor(out=blk[j][:], in0=b0t[:], in1=w[j][:], op=AO.mult)
            V.tensor_tensor(out=tmp1[:], in0=t_gp[:], in1=w[j - 1][:], op=AO.mult)
            V.tensor_tensor(out=blk[j][:], in0=blk[j][:], in1=tmp1[:], op=AO.add)
        V.tensor_tensor(out=blk[6][:], in0=t_gp[:], in1=w[5][:], op=AO.mult)

        # ------- transposes: natural [NP,128] -> rho-replicated [128,NCH] -------
        ident = pp.tile([128, 128], F32)
        nc.gpsimd.memset(ident[:], 1.0)
        nc.gpsimd.affine_select(out=ident[:], in_=ident[:], pattern=[[1, 128]],
                                compare_op=AO.is_equal, fill=0.0, base=0,
                                channel_multiplier=-1)
        # [128,32] with a 32x32 identity in every 32-row block (for transposes
        # whose stationary sits at partition base 32*g0)
        ident4 = pp.tile([128, 32], F32)
        nc.gpsimd.memset(ident4[:], 1.0)
        for g0 in range(4):
            nc.gpsimd.affine_select(out=ident4[32 * g0:32 * g0 + 32, :],
                                    in_=ident4[32 * g0:32 * g0 + 32, :],
                                    pattern=[[1, 32]], compare_op=AO.is_equal,
                                    fill=0.0, base=0, channel_multiplier=-1)

        # rep4[f, 32*q'+f'] = (f == f'): replicates a 32-row tile to 4 blocks
        rep4 = pp.tile([32, 128], F32)
        nc.gpsimd.memset(rep4[:], 1.0)
        for q in range(4):
            nc.gpsimd.affine_select(out=rep4[:, 32 * q:32 * q + 32],
                                    in_=rep4[:, 32 * q:32 * q + 32],
                                    pattern=[[1, 32]], compare_op=AO.is_equal,
                                    fill=0.0, base=0, channel_multiplier=-1)

        blkR = [pp.tile([128, NCH, 7], F32, name=f"blkR{b}", tag=f"blkR{b}")
                for b in range(B_LOC)]
        zfR = [pp.tile([128, NCH], F32, name=f"zfR{b}", tag=f"zfR{b}")
               for b in range(B_LOC)]
        xT4 = [pp.tile([128, NP], F32, name=f"xT4{b}", tag=f"xT4{b}")
               for b in range(B_LOC)]

        tmp32 = pp.tile([32, NP], F32)

        def repl_transpose(src_plane_ap, dst_ap):
            """[NP, 32] col-block -> [128, NP], rows replicated to 4 blocks.

            Plain transpose to [32, NP], then a matmul against the static
            rep4 stacked-identity broadcasts the 32 rows to all 4 blocks.
            """
            ps1 = psum_tr.tile([32, NP], F32, name="ps_t1", tag="ps_tr")
            nc.tensor.transpose(ps1[:], src_plane_ap, ident[:NP, :NP])
            V.tensor_copy(out=tmp32[:], in_=ps1[:])
            ps2 = psum_tr.tile([128, NP], F32, name="ps_t2", tag="ps_tr")
            nc.tensor.matmul(ps2[:], rep4[:], tmp32[:], start=True, stop=True)
            V.tensor_copy(out=dst_ap, in_=ps2[:])

        for b in range(B_LOC):
            for g0 in range(4):
                csl = slice(b * 128 + 32 * g0, b * 128 + 32 * g0 + 32)
                for j in range(7):
                    repl_transpose(
                        blk[j][:, csl],
                        blkR[b][:, :, j]
                        .rearrange("p (P gg) -> p P gg", gg=4)[:, :, g0],
                    )
                repl_transpose(
                    zf[:, csl],
                    zfR[b][:].rearrange("p (P gg) -> p P gg", gg=4)[:, :, g0],
                )
            ps = psum_tr.tile([128, NP], F32, name="ps_x", tag="ps_tr")
            nc.tensor.transpose(ps[:], nat_x[:, b * 128:(b + 1) * 128],
                                ident[:NP, :NP])
            V.tensor_copy(out=xT4[b][:], in_=ps[:])

        # ---------------- scatter index computation ----------------
        # partition = 32*rho + f ; v_j = zf - 3 + j - f ; fl = v//32 ; m = v%32
        # valid iff (c%4 + 3 - fl_j) % 4 == rho ; u16 idx = 64*(c%GS) + 2*(31-m_j)
        fi = pp.tile([128, 1], I32)
        nc.gpsimd.iota(fi[:], pattern=[[1, 1]], base=0, channel_multiplier=1)
        ff = pp.tile([128, 1], F32)
        V.tensor_copy(out=ff[:], in_=fi[:])
        s1 = pp.tile([128, 1], F32)
        s2 = pp.tile([128, 1], F32)
        s3 = pp.tile([128, 1], F32)
        i1 = pp.tile([128, 1], I32)
        V.tensor_scalar(out=s1[:], in0=ff[:], scalar1=1.0 / 32, scalar2=None, op0=AO.mult)
        V.tensor_copy(out=i1[:], in_=s1[:])
        V.tensor_copy(out=s2[:], in_=i1[:])
        V.tensor_tensor(out=s3[:], in0=s2[:], in1=s1[:], op=AO.is_gt)
        V.tensor_tensor(out=s2[:], in0=s2[:], in1=s3[:], op=AO.subtract)   # rho = p//32
        fmod = pp.tile([128, 1], F32)
        V.scalar_tensor_tensor(out=fmod[:], in0=s2[:], scalar=-32.0, in1=ff[:],
                               op0=AO.mult, op1=AO.add)                    # f = p%32
        sc0 = pp.tile([128, 1], F32)
        V.tensor_scalar(out=sc0[:], in0=fmod[:], scalar1=-1.0, scalar2=-3.0,
                        op0=AO.mult, op1=AO.add)                           # -3 - f
        rho_f = s2  # f32 [128,1] = p//32, kept for the is_equal scalar

        tcol = pp.tile([128, NCH], I16)
        nc.gpsimd.iota(tcol[:], pattern=[[0, NCH // 4], [1, 4]], base=0,
                       channel_multiplier=0)                               # c % 4
        scol = pp.tile([128, NCH, 14], I16)
        nc.gpsimd.iota(scol[:], pattern=[[0, NCH // GS], [64, GS], [0, 14]],
                       base=0, channel_multiplier=0)                       # 64*(c%GS)

        v0 = pp.tile([128, NCH], F32)
        tA = pp.tile([128, NCH], F32)
        tB = pp.tile([128, NCH], F32)
        tC = pp.tile([128, NCH], F32)
        fl_i = pp.tile([128, NCH], I16)
        m_i = pp.tile([128, NCH], I16)
        uB = pp.tile([128, NCH], I16)
        mneg = pp.tile([128, NCH], I16)
        iw = pp.tile([128, NCH], I16)
        iu = pp.tile([128, NCH], I16)
        iv = pp.tile([128, NCH], I16)

        idxR = [pp.tile([128, NCH, 14], I16, name=f"idxR{b}", tag=f"idxR{b}")
                for b in range(B_LOC)]

        for b in range(B_LOC):
            V.tensor_scalar(out=v0[:], in0=zfR[b][:], scalar1=sc0[:], scalar2=None,
                            op0=AO.add)
            V.tensor_scalar(out=tA[:], in0=v0[:], scalar1=1.0 / 32, scalar2=None,
                            op0=AO.mult)
            V.tensor_copy(out=fl_i[:], in_=tA[:])
            V.tensor_copy(out=tB[:], in_=fl_i[:])
            V.tensor_tensor(out=tC[:], in0=tB[:], in1=tA[:], op=AO.is_gt)
            V.tensor_tensor(out=tB[:], in0=tB[:], in1=tC[:], op=AO.subtract)  # fl0
            m0 = tA
            V.scalar_tensor_tensor(out=m0[:], in0=tB[:], scalar=-32.0, in1=v0[:],
                                   op0=AO.mult, op1=AO.add)                   # m0
            V.tensor_copy(out=fl_i[:], in_=tB[:])
            V.tensor_copy(out=m_i[:], in_=m0[:])
            # uB = (c%4) + 3 - fl0 ; mneg = 62 - 2*m0
            V.tensor_scalar(out=uB[:], in0=fl_i[:], scalar1=-1, scalar2=3,
                            op0=AO.mult, op1=AO.add)
            V.tensor_tensor(out=uB[:], in0=uB[:], in1=tcol[:], op=AO.add)
            V.tensor_scalar(out=mneg[:], in0=m_i[:], scalar1=-2, scalar2=62,
                            op0=AO.mult, op1=AO.add)
            for j in range(7):
                V.tensor_scalar(out=iw[:], in0=m_i[:], scalar1=32 - j,
                                scalar2=None, op0=AO.is_ge)                # wrap
                V.tensor_tensor(out=iu[:], in0=uB[:], in1=iw[:], op=AO.subtract)
                V.tensor_scalar(out=iv[:], in0=iu[:], scalar1=4, scalar2=None,
                                op0=AO.is_ge)
                V.scalar_tensor_tensor(out=iv[:], in0=iv[:], scalar=-4, in1=iu[:],
                                       op0=AO.mult, op1=AO.add)            # rho*
                V.tensor_scalar(out=iv[:], in0=iv[:], scalar1=rho_f[:],
                                scalar2=None, op0=AO.is_equal)             # valid
                V.scalar_tensor_tensor(out=iu[:], in0=iw[:], scalar=64, in1=mneg[:],
                                       op0=AO.mult, op1=AO.add)
                V.tensor_scalar(out=iu[:], in0=iu[:], scalar1=-2 * j - 20000,
                                scalar2=None, op0=AO.add)
                V.scalar_tensor_tensor(out=idxR[b][:, :, 2 * j], in0=iv[:],
                                       scalar=20000, in1=iu[:],
                                       op0=AO.mult, op1=AO.add)
                V.tensor_scalar(out=idxR[b][:, :, 2 * j + 1],
                                in0=idxR[b][:, :, 2 * j],
                                scalar1=1, scalar2=None, op0=AO.add)
            flat = idxR[b][:].rearrange("p c j -> p (c j)")
            sflat = scol[:].rearrange("p c j -> p (c j)")
            V.tensor_tensor(out=flat, in0=flat, in1=sflat, op=AO.add)

        # -------------- y ring (2 column-parity tiles per seq) --------------
        NRC = NP // 2 + 1
        ring = [[pp.tile([128, NRC], F32, name=f"ring{b}_{par}",
                         tag=f"ring{b}_{par}") for par in range(2)]
                for b in range(B_LOC)]
        for b in range(B_LOC):
            V.memset(ring[b][0][:, 0:1], 0.0)

        def ring_ap(b, nu, rows):
            return ring[b][nu % 2][rows, nu // 2:nu // 2 + 1]


        blkR_u16 = [blkR[b][:].bitcast(U16) for b in range(B_LOC)]

        # ---------------- sequential chain ----------------
        for g in range(NG):
            t4s = []
            for b in range(B_LOC):
                traw = traw_pool.tile([128, 32 * GS], F32, name="traw",
                                      tag=f"traw{b}")
                nc.gpsimd.local_scatter(
                    out_ap=traw[:].bitcast(U16),
                    data_ap=blkR_u16[b][:, GS * g:GS * (g + 1), :]
                    .rearrange("p c j -> p (c j)"),
                    idxs_ap=idxR[b][:, GS * g:GS * (g + 1), :]
                    .rearrange("p c j -> p (c j)"),
                    channels=128, num_elems=64 * GS, num_idxs=14 * GS,
                )
                t4 = t4_pool.tile([128, 32 * GS], F32, name="t4", tag=f"t4{b}")
                nc.vector.transpose(out=t4[:], in_=traw[:])
                t4s.append(t4)
            for s in range(GS):
                for b in range(B_LOC):
                    t4 = t4s[b]
                    c = GS * g + s
                    gg, t = c // 4, c % 4
                    ps = psum_c.tile([128, 1], F32, name=f"ps{b}", tag=f"ps{b}")
                    pieces = PIECES[t]
                    for i, (rb, rs, cd) in enumerate(pieces):
                        rows = slice(rb, rb + rs)
                        nc.tensor.matmul(
                            ps[32 * t:32 * t + 32, 0:1],
                            t4[rows, 32 * s:32 * s + 32],
                            ring_ap(b, gg + cd, rows),
                            start=(i == 0), stop=(i == len(pieces) - 1),
                            tile_position=(rb, 32 * t),
                        )
                    if s == 0:
                        # group boundary: DVE is busy with the t4 transpose;
                        # route this evac to ACT to keep the chain moving
                        nc.scalar.activation(
                            out=ring_ap(b, 1 + gg, slice(32 * t, 32 * t + 32)),
                            in_=ps[32 * t:32 * t + 32, 0:1],
                            func=AF.Identity,
                            bias=xT4[b][32 * t:32 * t + 32, gg:gg + 1],
                            scale=1.0,
                        )
                    else:
                        V.scalar_tensor_tensor(
                            out=ring_ap(b, 1 + gg, slice(32 * t, 32 * t + 32)),
                            in0=ps[32 * t:32 * t + 32, 0:1],
                            scalar=1.0, in1=xT4[b][32 * t:32 * t + 32, gg:gg + 1],
                            op0=AO.mult, op1=AO.add,
                        )

        # ---------------- output transpose + store ----------------
        NH = NP // 2
        for b in range(B_LOC):
            for par in range(2):
                # parity tile local col l holds block P = 2*l + par - 1
                ynat = pp.tile([NH, 128], F32, name=f"ynat{b}{par}",
                               tag=f"ynat{b}{par}")
                for g0 in range(4):
                    src = ring[b][par][32 * g0:32 * g0 + 32,
                                       1 - par:1 - par + NH]
                    ps = psum_tr.tile([NH, 32], F32, name="ps_out", tag="ps_tr")
                    nc.tensor.transpose(ps[:, :], src,
                                        ident4[32 * g0:32 * g0 + 32, :],
                                        tile_position=(32 * g0, 0))
                    V.tensor_copy(out=ynat[:, 32 * g0:32 * g0 + 32], in_=ps[:, :])
                nc.sync.dma_start(
                    out=out_d[b].rearrange("(P j) -> P j", j=128)[1 - par::2],
                    in_=ynat[:],
                )


def build_program(N=N_FULL):
    nc = bacc.Bacc("TRN2", target_bir_lowering=False, debug=False,
                   enable_asserts=False)
    f0_d = nc.dram_tensor("f0", [B_LOC, N], F32, kind="ExternalInput").ap()
    x_d = nc.dram_tensor("x", [B_LOC, N], F32, kind="ExternalInput").ap()
    lb_d = nc.dram_tensor("l_b", [B_LOC, N, 2], F32, kind="ExternalInput").ap()
    out_d = nc.dram_tensor("out", [B_LOC, N], F32, kind="ExternalOutput").ap()
    with tile.TileContext(nc) as tc:
        build_kernel(tc, out_d, f0_d, x_d, lb_d, N)
    nc.compile()
    return nc


_PROGRAM_CACHE = {}


def _get_program(N=N_FULL):
    if N not in _PROGRAM_CACHE:
        _PROGRAM_CACHE[N] = build_program(N)
    return _PROGRAM_CACHE[N]


def kernel(f0, x, l_b, K=108, **kwargs):
    """Full-input entry point: shards batch across 8 cores, returns full output."""
    f0 = np.asarray(f0, dtype=np.float32)
    x = np.asarray(x, dtype=np.float32)
    l_b = np.asarray(l_b, dtype=np.float32)
    B, N = x.shape
    assert B == B_FULL and int(K) == 108
    nc = _get_program(N)
    in_maps = []
    for i in range(NCORES):
        sl = slice(i * B_LOC, (i + 1) * B_LOC)
        in_maps.append({
            "f0": np.ascontiguousarray(f0[sl]),
            "x": np.ascontiguousarray(x[sl]),
            "l_b": np.ascontiguousarray(l_b[sl]),
        })
    res = bass_utils.run_bass_kernel_spmd(nc, in_maps, core_ids=list(range(NCORES)))
    out = np.concatenate([res.results[i]["out"] for i in range(NCORES)], axis=0)
    return out.astype(np.float32)

